# revision 20
# baseline (speedup 1.0000x reference)
"""Trainium2 Bass kernel: GQA attention block (S=2048, HID=4096, 32 q heads /
8 kv heads, head dim 128, RoPE, causal), tensor-parallel over heads on 8
NeuronCores.

Sharding: core c owns q heads [4c..4c+4) and kv head c. wq/wk/wv are sharded
on their output dim, wo on its input dim; each core computes a partial
y_c = o_c @ wo_c.T and the host sums the 8 bf16 partials (the "all-reduce").

Fully software-pipelined bf16 kernel (~436us vs 521us f32r baseline):
  - All matmul operands bf16: 1 cycle/row at 2.4 GHz on the PE. (f32r runs
    a two-pass LOW/HIGH scheme at ~1.28 cycles/row, measured from traces.)
  - The sequence is processed in 4 chunks of 512. Per chunk: phase 1
    projects x to q/k/v (RoPE applied during the PSUM drain), phase 2 runs
    causal flash-style attention over key blocks, phase 3 multiplies by wo.
  - Phases are software-pipelined at instruction granularity: p1(c+1) and
    p3(c-1) are emitted as small "filler" closures between p2(c)
    j-iterations so the PE never starves on the scores->exp->mask->PV
    dependency chain. PSUM budget: p1 2 banks + p2 4 + p3 2 = 8.
  - Matmuls never accumulate back-to-back into the same PSUM bank (costs
    ~56ns/matmul); accumulations alternate between two banks everywhere.
  - Softmax denominator off the PE: DVE accumulates den_acc += et_j per
    head (bf16), then one 512-row ones-matmul reduces it, DVE takes the
    reciprocal of the [1,512] row, and a second 512-row matmul broadcasts
    it to 128 partitions (no DRAM round-trip). The broadcast+normalize is
    deferred into the next pair's filler stream to hide the DVE chain.
  - Causal masking of diagonal 128-blocks is a DVE multiply with a
    precomputed triangular 0/1 tile (keeps GpSimd off the critical chain).
  - DMA: weights/x arrive as one dma_start per small tile (descriptor issue
    costs ~650ns each on the issuing engine); x tiles are issued from the
    otherwise-idle GpSimd queue; weights stream just-in-time during chunk 0.
  - yT output in bf16 (halves the 32MB writeback); wo resident in SBUF.
"""

import os
import sys

import numpy as np

for _p in (
    "/root/.axon_site",
    "/root/.axon_site/_ro/trn_rl_repo",
    "/root/.axon_site/_ro/pypackages",
    "/opt/trn_rl_repo",
):
    if os.path.isdir(_p) and _p not in sys.path:
        sys.path.append(_p)

import concourse.bacc as bacc  # noqa: E402
import concourse.mybir as mybir  # noqa: E402
from concourse import bass_utils  # noqa: E402
from concourse.tile import TileContext  # noqa: E402

F32 = mybir.dt.float32
F32R = mybir.dt.float32r
BF16 = mybir.dt.bfloat16

N_CORES = 8
SEQ = 2048
HID = 4096
NQ = 32
NKV = 8
HD = 128
THETA = 500000.0

HQ = NQ // N_CORES  # 4 q heads per core
QC = HQ * HD  # 512: per-core q feature slice
NKBLK = HID // 128  # 32 contraction blocks for the projections
NCHUNK = SEQ // 512  # 4 sequence chunks of 512
SCALE = 1.0 / float(np.sqrt(HD))

MODE = "bf16"


def _build_body(tc, sb, sbw, ps):
    nc = tc.nc
    mdt = BF16

    xT = nc.dram_tensor("xT", (HID, SEQ), mdt, kind="ExternalInput").ap()
    wq_sb_d = nc.dram_tensor("wq_sb", (128, NKBLK * QC), mdt, kind="ExternalInput").ap()
    wk_sb_d = nc.dram_tensor("wk_sb", (128, NKBLK * HD), mdt, kind="ExternalInput").ap()
    wv_sb_d = nc.dram_tensor("wv_sb", (128, NKBLK * HD), mdt, kind="ExternalInput").ap()
    wo_sb_d = nc.dram_tensor("wo_sb", (128, 4 * HID), mdt, kind="ExternalInput").ap()
    cc_d = nc.dram_tensor("cc", (HD, SEQ), F32, kind="ExternalInput").ap()
    ss_d = nc.dram_tensor("ss", (HD, SEQ), F32, kind="ExternalInput").ap()
    yT_d = nc.dram_tensor("yT", (HID, SEQ), mdt, kind="ExternalOutput").ap()
    dscr = nc.dram_tensor("den_scratch", (1, 512), F32).ap()

    # --- persistent SBUF tiles ---
    ones_f = sb.tile([128, 128], F32, name="ones_f")
    nc.vector.memset(ones_f[:], 1.0)
    ones = sb.tile([128, 128], F32R, name="ones")
    nc.vector.tensor_copy(ones[:], ones_f[:])
    ones_b = sb.tile([128, 128], BF16, name="ones_b")
    nc.vector.tensor_copy(ones_b[:], ones_f[:])

    # resident weights, split into small tiles so the tile-level dependency
    # tracking lets the first matmuls start after the first small DMA.
    # wq: 8 tiles of 4 k-blocks; wk/wv: 4 tiles of 8 k-blocks; wo: 1 tile.
    wq_t = [sb.tile([128, 4 * QC], mdt, name=f"wq_t{g}") for g in range(8)]
    wk_t = [sb.tile([128, 8 * HD], mdt, name=f"wk_t{g}") for g in range(4)]
    wv_t = [sb.tile([128, 8 * HD], mdt, name=f"wv_t{g}") for g in range(4)]
    wo_t = sb.tile([128, 4 * HID], mdt, name="wo_t")
    def load_wk_wv(g):
        nc.sync.dma_start(wk_t[g][:], wk_sb_d[:, 8 * g * HD : (8 * g + 8) * HD])
        nc.sync.dma_start(wv_t[g][:], wv_sb_d[:, 8 * g * HD : (8 * g + 8) * HD])

    def load_wq(g):
        nc.sync.dma_start(wq_t[g][:], wq_sb_d[:, 4 * g * QC : (4 * g + 4) * QC])

    def wo_units():
        units = []

        def mk(i):
            def run():
                w = 4 * HID // 4
                nc.sync.dma_start(wo_t[:, i * w : (i + 1) * w], wo_sb_d[:, i * w : (i + 1) * w])
            return run

        return [mk(i) for i in range(4)]

    # PE warmup: dummy matmuls so the HAM clock gate opens before the first
    # real matmul; kept alive by a tiny DMA into the scratch tensor.
    warm_in = sb.tile([128, 128], F32, name="warm_in")
    nc.vector.memset(warm_in[:], 0.5)
    warm_ps = ps.tile([128, 128], F32, tag="p1", bufs=2, name="warm_ps")
    for wi in range(16):
        nc.tensor.matmul(warm_ps[:], warm_in[:], warm_in[:], start=(wi == 0), stop=(wi == 15))
    warm_sb = sbw.tile([1, 128], F32, tag="den", bufs=4, name="warm_sb")
    nc.vector.tensor_copy(warm_sb[0:1, :], warm_ps[0:1, :])
    nc.sync.dma_start(dscr[0:1, 0:128], warm_sb[0:1, :])

    # per-chunk tensors: q (reused as normalized o after p2), k, v-natural
    qT = [[sb.tile([128, 512], mdt, name=f"qT{c}_{h}") for h in range(HQ)] for c in range(NCHUNK)]
    kT = [sb.tile([128, 512], mdt, name=f"kT{c}") for c in range(NCHUNK)]
    vnat = [sb.tile([128, 512], mdt, name=f"vnat{c}") for c in range(NCHUNK)]

    ident = sb.tile([128, 128], mdt, name="ident")
    from concourse.masks import make_identity

    make_identity(nc, ident)

    # lower-triangular (keys kk <= s) 0/1 mask for the diagonal 128-blocks
    tri = sb.tile([128, 128], mdt, name="tri")
    nc.vector.memset(tri[:], 1.0)
    nc.gpsimd.affine_select(
        out=tri[:],
        in_=tri[:],
        compare_op=mybir.AluOpType.is_ge,
        fill=0.0,
        base=0,
        pattern=[[1, 128]],
        channel_multiplier=-1,
    )

    # =================== phase 1: QKV projections + RoPE ===================
    def rope_inplace(dst, psrc, cct, sst):
        """dst[:, 0:512] = rope(psrc); partition rows 0:64 hold the even rope
        dims, 64:128 the odd ones (host permuted the weight rows)."""
        cpy = sbw.tile([128, 512], F32, tag="ropetmp", bufs=5, name="cpy")
        nc.scalar.copy(cpy[:], psrc[:])
        sw = sbw.tile([128, 512], F32, tag="ropetmp", bufs=5, name="sw")
        nc.scalar.copy(sw[0:64, :], cpy[64:128, :])
        nc.scalar.copy(sw[64:128, :], cpy[0:64, :])
        m1 = sbw.tile([128, 512], F32, tag="ropetmp", bufs=5, name="m1")
        m2 = sbw.tile([128, 512], F32, tag="ropetmp", bufs=5, name="m2")
        nc.gpsimd.tensor_mul(m1[:], cpy[:], cct[:])
        nc.gpsimd.tensor_mul(m2[:], sw[:], sst[:])
        nc.vector.tensor_sub(dst[0:64, :], m1[0:64, :], m2[0:64, :])
        nc.vector.tensor_add(dst[64:128, :], m1[64:128, :], m2[64:128, :])

    def p1_units(c, pair_order=((0, 1), (2, 3), (4, 5)), inject_weights=False):
        """Phase-1 filler closures for chunk c, as a list of per-pair unit
        lists. Outputs 0-3 are q heads, 4 is k, 5 is v. Each pair alternates
        two PSUM banks (same-bank back-to-back accumulation can't pipeline)."""
        s0 = c * 512
        # chunk-resident x: 8 tiles of 4 k-blocks [128, 4*512]
        xg = [None] * 8

        def load_xg(g):
            def run():
                xg[g] = sbw.tile([128, 4 * 512], mdt, tag="xt", bufs=12, name=f"xg{c}_{g}")
                nc.gpsimd.dma_start(
                    xg[g][:].rearrange("p (k s) -> p k s", k=4),
                    xT.rearrange("(k p) s -> p k s", p=128)[:, 4 * g : 4 * g + 4, s0 : s0 + 512],
                )
            return run

        cs = [None, None]

        def load_tbl():
            def run():
                cs[0] = sbw.tile([128, 512], F32, tag="tbl", bufs=4, name="cct")
                cs[1] = sbw.tile([128, 512], F32, tag="tbl", bufs=4, name="sst")
                nc.sync.dma_start(cs[0][:], cc_d[:, s0 : s0 + 512])
                nc.sync.dma_start(cs[1][:], ss_d[:, s0 : s0 + 512])
            return run

        state = {}

        def mk_mm(out_i, k):
            def run():
                if k == 0:
                    state[out_i] = ps.tile([128, 512], F32, tag="p1", bufs=2, name=f"p1acc{out_i}")
                if out_i < HQ:
                    wsl = wq_t[k // 4][:, (k % 4) * QC + out_i * 128 : (k % 4) * QC + (out_i + 1) * 128]
                elif out_i == HQ:
                    wsl = wk_t[k // 8][:, (k % 8) * HD : (k % 8 + 1) * HD]
                else:
                    wsl = wv_t[k // 8][:, (k % 8) * HD : (k % 8 + 1) * HD]
                xsl = xg[k // 4][:, (k % 4) * 512 : (k % 4 + 1) * 512]
                nc.tensor.matmul(state[out_i][:], wsl, xsl, start=(k == 0), stop=(k == NKBLK - 1))
            return run

        def mk_drain(out_i):
            def run():
                if out_i < HQ:
                    rope_inplace(qT[c][out_i], state[out_i], cs[0], cs[1])
                elif out_i == HQ:
                    rope_inplace(kT[c], state[out_i], cs[0], cs[1])
                else:
                    vtmp = sbw.tile([128, 512], mdt, tag="ropetmp", bufs=5, name="vtmp")
                    nc.scalar.copy(vtmp[:], state[out_i][:])
                    for i in range(4):
                        tp = ps.tile([128, 128], mdt, tag="p1", bufs=2, name="tp")
                        nc.tensor.transpose(tp[:], vtmp[:, i * 128 : (i + 1) * 128], ident)
                        nc.scalar.copy(vnat[c][:, i * 128 : (i + 1) * 128], tp[:])
            return run

        pair_lists = []
        loaded_xg = {0}
        for pi, (a, b) in enumerate(pair_order):
            units = []
            if pi == 0:
                units.append(load_xg(0))
                units.append(load_tbl())
            for k in range(NKBLK):
                if k % 4 == 0 and pi == 0:
                    for gg in (k // 4 + 1, k // 4 + 2):
                        if gg < 8 and gg not in loaded_xg:
                            loaded_xg.add(gg)
                            units.append(load_xg(gg))
                if inject_weights and pi == 0:
                    # chunk 0 streams weights just-in-time: wk/wv one 8-block
                    # tile ahead, wq tiles spread across the first pair.
                    if k % 8 == 0 and k // 8 + 1 < 4:
                        units.append(lambda g=k // 8 + 1: load_wk_wv(g))
                    if k % 4 == 2:
                        units.append(lambda g=k // 4: load_wq(g))
                units.append(mk_mm(a, k))
                units.append(mk_mm(b, k))
            units.append(mk_drain(a))
            units.append(mk_drain(b))
            pair_lists.append(units)
        return pair_lists

    # =================== phase 3: output projection ===================
    def p3_units(c, wide_banks=False):
        """Phase-3 filler closures for chunk c: 8 groups of 4 jb each. With
        wide_banks (p1 banks idle), rotate y_ps over 4 PSUM banks."""
        units = []

        def mk_group(jbg):
            def run():
                yst = sbw.tile([128, 4 * 512], mdt, tag="yst", bufs=2, name="yst")
                for jp in range(2):
                    tag = "p1" if (wide_banks and jp == 1) else "p3"
                    y_ps = {}
                    for jj in (2 * jp, 2 * jp + 1):
                        y_ps[jj] = ps.tile([128, 512], F32, tag=tag, bufs=2, name="y_ps")
                    for cb in range(4):
                        for jj in (2 * jp, 2 * jp + 1):
                            jb = jbg * 4 + jj
                            nc.tensor.matmul(
                                y_ps[jj][:],
                                wo_t[:, cb * HID + jb * 128 : cb * HID + (jb + 1) * 128],
                                qT[c][cb][:],
                                start=(cb == 0),
                                stop=(cb == 3),
                            )
                    for jj in (2 * jp, 2 * jp + 1):
                        if jj % 2 == 0:
                            nc.scalar.copy(yst[:, jj * 512 : (jj + 1) * 512], y_ps[jj][:])
                        else:
                            nc.vector.tensor_copy(yst[:, jj * 512 : (jj + 1) * 512], y_ps[jj][:])
                nc.sync.dma_start(
                    yT_d.rearrange("(jb p) s -> p jb s", p=128)[
                        :, jbg * 4 : jbg * 4 + 4, c * 512 : (c + 1) * 512
                    ],
                    yst[:].rearrange("p (jb s) -> p jb s", jb=4),
                )
            return run

        for jbg in range(8):
            units.append(mk_group(jbg))
        return units

    # =================== phase 2: attention ===================
    def p2_chunk(c, fillers):
        """Attention for chunk c, pulling filler units between j-iterations.
        Returns the last pair's drain units for the next chunk's stream."""
        carry = []
        jmax = 4 * c + 3
        n_slots = 2 * (jmax + 2)
        total = len(fillers)
        slot = [0, 0]  # slots done, units consumed

        def fill():
            slot[0] += 1
            target = (total * slot[0]) // n_slots if slot[0] < n_slots else total
            while slot[1] < target and fillers:
                fillers.pop(0)()
                slot[1] += 1

        for hp in range(HQ // 2):
            heads = (2 * hp, 2 * hp + 1)
            o_ps = {h: ps.tile([128, 512], F32, tag="o", bufs=2, name=f"o_ps{h}") for h in heads}
            den_acc = {}
            for h in heads:
                den_acc[h] = sbw.tile([128, 512], mdt, tag="den_acc", bufs=3, name=f"den_acc{h}")
            for j in range(jmax + 1):
                off = 128 * max(0, j - 4 * c)
                g = j - 4 * c
                st = j == 0
                sp = j == jmax
                jc, jb2 = j // 4, j % 4
                et = {}
                s_ps = {}
                for h in heads:
                    s_ps[h] = ps.tile([128, 512], F32, tag="s", bufs=2, name="s_ps")
                    nc.tensor.matmul(
                        s_ps[h][:, off:512],
                        kT[jc][:, jb2 * 128 : (jb2 + 1) * 128],
                        qT[c][h][:, off:512],
                        start=True,
                        stop=True,
                    )
                for h in heads:
                    et[h] = sbw.tile([128, 512], mdt, tag="et", bufs=10, name="et")
                    nc.scalar.activation(
                        et[h][:, off:512], s_ps[h][:, off:512],
                        mybir.ActivationFunctionType.Exp, scale=SCALE,
                    )
                    if g >= 0:  # diagonal block: keep keys kk <= s in block
                        nc.vector.tensor_mul(
                            et[h][:, g * 128 : (g + 1) * 128],
                            et[h][:, g * 128 : (g + 1) * 128],
                            tri[:],
                        )
                for h in heads:
                    nc.tensor.matmul(
                        o_ps[h][:, off:512],
                        vnat[jc][:, jb2 * 128 : (jb2 + 1) * 128],
                        et[h][:, off:512],
                        start=st,
                        stop=sp,
                    )
                for h in heads:
                    # j == 0 always has off == 0, so the copy initializes all
                    # 512 columns of the accumulator.
                    if j == 0:
                        nc.vector.tensor_copy(den_acc[h][:], et[h][:])
                    else:
                        nc.vector.tensor_add(
                            den_acc[h][:, off:512],
                            den_acc[h][:, off:512],
                            et[h][:, off:512],
                        )
                fill()
            # drain: o_sb copies + den matmuls + reciprocal chain inline;
            # the broadcast matmul + normalize mul are deferred into the next
            # pair's / chunk's filler stream so the PE never waits on the
            # DVE reciprocal chain.
            o_sb = {}
            for i, h in enumerate(heads):
                o_sb[h] = sbw.tile([128, 512], F32, tag="osb", bufs=3, name="o_sb")
                if i % 2 == 0:
                    nc.vector.tensor_copy(o_sb[h][:], o_ps[h][:])
                else:
                    nc.scalar.copy(o_sb[h][:], o_ps[h][:])
            fill()
            rec = {}
            for h in heads:
                den_ps = ps.tile([128, 512], F32, tag="s", bufs=2, name="den_ps")
                nc.tensor.matmul(
                    den_ps[0:1, :], ones_b[:, 0:1], den_acc[h][:], start=True, stop=True
                )
                den_row = sbw.tile([1, 512], F32, tag="den", bufs=4, name="den_row")
                nc.vector.tensor_copy(den_row[0:1, :], den_ps[0:1, :])
                rec_f = sbw.tile([1, 512], F32, tag="den", bufs=4, name="rec_f")
                rec_scr = sbw.tile([1, 512], F32, tag="den", bufs=4, name="rec_scr")
                nc.vector.reciprocal_approx_accurate(
                    rec_f[0:1, :], den_row[0:1, :], rec_scr[0:1, :]
                )
                rec[h] = sbw.tile([1, 512], F32R, tag="den", bufs=4, name="rec_row")
                nc.vector.tensor_copy(rec[h][0:1, :], rec_f[0:1, :])

            def mk_u2(heads=heads, o_sb=o_sb, rec=rec):
                def run():
                    rec_ps = {}
                    for h in heads:
                        rec_ps[h] = ps.tile([128, 512], F32, tag="s", bufs=2, name="rec_ps")
                        nc.tensor.matmul(
                            rec_ps[h][:], ones[0:1, :], rec[h][0:1, :],
                            start=True, stop=True,
                        )
                    for h in heads:
                        # qT[c][h] becomes the normalized attention output
                        nc.vector.tensor_mul(qT[c][h][:], o_sb[h][:], rec_ps[h][:])
                return run

            if hp == 0:
                fillers.insert(5, mk_u2())
            else:
                carry.append(mk_u2())
        return carry

    # =================== pipeline driver ===================
    # p1(0): (k,v) and (q0,q1) run straight; (q2,q3) become leading fillers
    # for p2(0) so its first pair can start as soon as q0/q1 are roped.
    load_wk_wv(0)
    p10 = p1_units(0, pair_order=((4, 5), (0, 1), (2, 3)), inject_weights=True)
    for u in p10[0]:
        u()
    for u in p10[1]:
        u()
    carry = []
    for c in range(NCHUNK):
        fillers = []
        fillers += carry
        if c == 0:
            fillers += p10[2]
            fillers += wo_units()
        if c + 1 < NCHUNK:
            for pl in p1_units(c + 1):
                fillers += pl
        if c - 1 >= 0:
            fillers += p3_units(c - 1, wide_banks=(c - 1 >= 2))
        carry = p2_chunk(c, fillers)
        for u in fillers:  # anything the pacing didn't consume
            u()
    for u in carry:
        u()
    for u in p3_units(NCHUNK - 1, wide_banks=True):
        u()


_NC_CACHE = {}


def _get_nc():
    key = ("v2", MODE)
    if key not in _NC_CACHE:
        nc = bacc.Bacc("TRN2", target_bir_lowering=False, debug=False, num_devices=N_CORES)
        with TileContext(nc) as tc:
            with (
                tc.tile_pool(name="sb", bufs=1) as sb,
                tc.tile_pool(name="sbw", bufs=1) as sbw,
                tc.tile_pool(name="ps", bufs=1, space="PSUM") as ps,
            ):
                _build_body(tc, sb, sbw, ps)
        nc.compile()
        _NC_CACHE[key] = nc
    return _NC_CACHE[key]


_ROPE_PERM = np.concatenate([np.arange(0, 128, 2), np.arange(1, 128, 2)])


def _rope_tables(start_pos):
    freqs = 1.0 / (THETA ** (np.arange(0, HD, 2, dtype=np.float64) / HD))
    t = np.arange(start_pos, start_pos + SEQ, dtype=np.float64)
    ang = np.outer(t, freqs)  # [SEQ, 64]
    cosT = np.cos(ang).T.astype(np.float32)  # [64, SEQ]
    sinT = np.sin(ang).T.astype(np.float32)
    cc = np.ascontiguousarray(np.concatenate([cosT, cosT], axis=0))
    ss = np.ascontiguousarray(np.concatenate([sinT, sinT], axis=0))
    return cc, ss


def _to_kblock_layout(wT, cwidth):
    """[HID, cwidth] feature-major weight -> [128, NKBLK*cwidth] with k-block
    k at columns [k*cwidth, (k+1)*cwidth)."""
    return np.ascontiguousarray(
        wT.reshape(NKBLK, 128, cwidth).transpose(1, 0, 2).reshape(128, NKBLK * cwidth)
    )


def make_in_maps(x, wq, wk, wv, wo, start_pos):
    import ml_dtypes

    np_mdt = ml_dtypes.bfloat16
    x = np.asarray(x, dtype=np.float32)
    wq = np.asarray(wq, dtype=np.float32)
    wk = np.asarray(wk, dtype=np.float32)
    wv = np.asarray(wv, dtype=np.float32)
    wo = np.asarray(wo, dtype=np.float32)
    sp = int(start_pos)

    xT = np.ascontiguousarray(x.T).astype(np_mdt, copy=False)
    cc, ss = _rope_tables(sp)
    woT = np.ascontiguousarray(wo.T)  # [in=c, out=j]

    in_maps = []
    for c in range(N_CORES):
        wq_c = wq[c * QC : (c + 1) * QC, :]  # [512, HID]
        wq_c = wq_c.reshape(HQ, HD, HID)[:, _ROPE_PERM, :].reshape(QC, HID)
        wk_c = wk[c * HD : (c + 1) * HD, :][_ROPE_PERM, :]  # [128, HID]
        wv_c = wv[c * HD : (c + 1) * HD, :]  # [128, HID]
        wq_sbm = _to_kblock_layout(np.ascontiguousarray(wq_c.T), QC)
        wk_sbm = _to_kblock_layout(np.ascontiguousarray(wk_c.T), HD)
        wv_sbm = _to_kblock_layout(np.ascontiguousarray(wv_c.T), HD)
        woT_c = woT[c * QC : (c + 1) * QC, :]  # [512, HID]
        wo_sbm = np.ascontiguousarray(
            woT_c.reshape(4, 128, HID).transpose(1, 0, 2).reshape(128, 4 * HID)
        )
        in_maps.append(
            {
                "xT": xT,
                "wq_sb": wq_sbm.astype(np_mdt, copy=False),
                "wk_sb": wk_sbm.astype(np_mdt, copy=False),
                "wv_sb": wv_sbm.astype(np_mdt, copy=False),
                "wo_sb": wo_sbm.astype(np_mdt, copy=False),
                "cc": cc,
                "ss": ss,
            }
        )
    return in_maps


def _assemble(results):
    acc = results[0]["yT"].astype(np.float32)
    for r in results[1:]:
        acc = acc + r["yT"].astype(np.float32)
    return np.ascontiguousarray(acc.T)


def _row0_expected(x, wv, wo):
    """Exact y[0]: query 0 attends only key 0, so o[0] is v[0] broadcast over
    the 4 q heads of each kv head; cheap host-side corruption check."""
    v0 = np.asarray(x[0], np.float64) @ np.asarray(wv, np.float64).T  # [1024]
    o0 = np.concatenate([v0[(h // HQ) * HD : (h // HQ + 1) * HD] for h in range(NQ)])
    return o0 @ np.asarray(wo, np.float64).T  # [4096]


def kernel(x, wq, wk, wv, wo, start_pos):
    nc = _get_nc()
    in_maps = make_in_maps(x, wq, wk, wv, wo, start_pos)
    y0 = _row0_expected(x, wv, wo)
    out = None
    for attempt in range(2):
        res = bass_utils.run_bass_kernel_spmd(nc, in_maps, core_ids=list(range(N_CORES)))
        out = _assemble(res.results)
        err0 = float(np.linalg.norm(out[0] - y0) / (np.linalg.norm(y0) + 1e-30))
        if np.isfinite(out).all() and err0 < 2e-2:
            break
        # a wedged device can corrupt a run silently; one retry clears it
    return out


# revision 21
# speedup vs baseline: 1.1977x; 1.1977x over previous
"""Trainium2 Bass kernel: GQA attention block (S=2048, HID=4096, 32 q heads /
8 kv heads, head dim 128, RoPE, causal), tensor-parallel over heads on 8
NeuronCores.

Sharding: core c owns q heads [4c..4c+4) and kv head c. wq/wk/wv are sharded
on their output dim, wo on its input dim; each core computes a partial
y_c = o_c @ wo_c.T and the host sums the 8 bf16 partials (the "all-reduce").

Fully software-pipelined bf16 kernel (~436us vs 521us f32r baseline):
  - All matmul operands bf16: 1 cycle/row at 2.4 GHz on the PE. (f32r runs
    a two-pass LOW/HIGH scheme at ~1.28 cycles/row, measured from traces.)
  - The sequence is processed in 4 chunks of 512. Per chunk: phase 1
    projects x to q/k/v (RoPE applied during the PSUM drain), phase 2 runs
    causal flash-style attention over key blocks, phase 3 multiplies by wo.
  - Phases are software-pipelined at instruction granularity: p1(c+1) and
    p3(c-1) are emitted as small "filler" closures between p2(c)
    j-iterations so the PE never starves on the scores->exp->mask->PV
    dependency chain. PSUM budget: p1 2 banks + p2 4 + p3 2 = 8.
  - Matmuls never accumulate back-to-back into the same PSUM bank (costs
    ~56ns/matmul); accumulations alternate between two banks everywhere.
  - Softmax denominator off the PE: DVE accumulates den_acc += et_j per
    head (bf16), then one 512-row ones-matmul reduces it, DVE takes the
    reciprocal of the [1,512] row, and a second 512-row matmul broadcasts
    it to 128 partitions (no DRAM round-trip). The broadcast+normalize is
    deferred into the next pair's filler stream to hide the DVE chain.
  - Causal masking of diagonal 128-blocks is a GpSimd affine_select on
    the exp output (runs off the DVE/PE critical engines).
  - DMA: weights/x arrive as one dma_start per small tile (descriptor issue
    costs ~650ns each on the issuing engine); x tiles are issued from the
    otherwise-idle GpSimd queue; weights stream just-in-time during chunk 0.
  - yT output in bf16 (halves the 32MB writeback); wo resident in SBUF.
"""

import os
import sys

import numpy as np

for _p in (
    "/root/.axon_site",
    "/root/.axon_site/_ro/trn_rl_repo",
    "/root/.axon_site/_ro/pypackages",
    "/opt/trn_rl_repo",
):
    if os.path.isdir(_p) and _p not in sys.path:
        sys.path.append(_p)

import concourse.bacc as bacc  # noqa: E402
import concourse.mybir as mybir  # noqa: E402
from concourse import bass_utils  # noqa: E402
from concourse.tile import TileContext  # noqa: E402

F32 = mybir.dt.float32
F32R = mybir.dt.float32r
BF16 = mybir.dt.bfloat16

N_CORES = 8
SEQ = 2048
HID = 4096
NQ = 32
NKV = 8
HD = 128
THETA = 500000.0

HQ = NQ // N_CORES  # 4 q heads per core
QC = HQ * HD  # 512: per-core q feature slice
NKBLK = HID // 128  # 32 contraction blocks for the projections
NCHUNK = SEQ // 512  # 4 sequence chunks of 512
SCALE = 1.0 / float(np.sqrt(HD))

MODE = "bf16"


def _build_body(tc, sb, sbw, ps):
    nc = tc.nc
    mdt = BF16

    xT = nc.dram_tensor("xT", (HID, SEQ), mdt, kind="ExternalInput").ap()
    wq_sb_d = nc.dram_tensor("wq_sb", (128, NKBLK * QC), mdt, kind="ExternalInput").ap()
    wk_sb_d = nc.dram_tensor("wk_sb", (128, NKBLK * HD), mdt, kind="ExternalInput").ap()
    wv_sb_d = nc.dram_tensor("wv_sb", (128, NKBLK * HD), mdt, kind="ExternalInput").ap()
    wo_sb_d = nc.dram_tensor("wo_sb", (128, 4 * HID), mdt, kind="ExternalInput").ap()
    cc_d = nc.dram_tensor("cc", (HD, SEQ), F32, kind="ExternalInput").ap()
    ss_d = nc.dram_tensor("ss", (HD, SEQ), F32, kind="ExternalInput").ap()
    yT_d = nc.dram_tensor("yT", (HID, SEQ), mdt, kind="ExternalOutput").ap()
    dscr = nc.dram_tensor("den_scratch", (1, 512), F32).ap()

    # --- persistent SBUF tiles ---
    ones_f = sb.tile([128, 128], F32, name="ones_f")
    nc.vector.memset(ones_f[:], 1.0)
    ones = sb.tile([128, 128], F32R, name="ones")
    nc.vector.tensor_copy(ones[:], ones_f[:])
    ones_b = sb.tile([128, 128], BF16, name="ones_b")
    nc.vector.tensor_copy(ones_b[:], ones_f[:])

    # resident weights, split into small tiles so the tile-level dependency
    # tracking lets the first matmuls start after the first small DMA.
    # wq: 8 tiles of 4 k-blocks; wk/wv: 4 tiles of 8 k-blocks; wo: 1 tile.
    wq_t = [sb.tile([128, 4 * QC], mdt, name=f"wq_t{g}") for g in range(8)]
    wk_t = [sb.tile([128, 8 * HD], mdt, name=f"wk_t{g}") for g in range(4)]
    wv_t = [sb.tile([128, 8 * HD], mdt, name=f"wv_t{g}") for g in range(4)]
    wo_t = sb.tile([128, 4 * HID], mdt, name="wo_t")
    def load_wk_wv(g):
        nc.sync.dma_start(wk_t[g][:], wk_sb_d[:, 8 * g * HD : (8 * g + 8) * HD])
        nc.sync.dma_start(wv_t[g][:], wv_sb_d[:, 8 * g * HD : (8 * g + 8) * HD])

    def load_wq(g):
        nc.sync.dma_start(wq_t[g][:], wq_sb_d[:, 4 * g * QC : (4 * g + 4) * QC])

    def wo_units():
        units = []

        def mk(i):
            def run():
                w = 4 * HID // 4
                nc.sync.dma_start(wo_t[:, i * w : (i + 1) * w], wo_sb_d[:, i * w : (i + 1) * w])
            return run

        return [mk(i) for i in range(4)]

    # PE warmup: dummy matmuls so the HAM clock gate opens before the first
    # real matmul; kept alive by a tiny DMA into the scratch tensor.
    warm_in = sb.tile([128, 128], F32, name="warm_in")
    nc.vector.memset(warm_in[:], 0.5)
    warm_ps = ps.tile([128, 128], F32, tag="p1", bufs=2, name="warm_ps")
    for wi in range(16):
        nc.tensor.matmul(warm_ps[:], warm_in[:], warm_in[:], start=(wi == 0), stop=(wi == 15))
    warm_sb = sbw.tile([1, 128], F32, tag="den", bufs=4, name="warm_sb")
    nc.vector.tensor_copy(warm_sb[0:1, :], warm_ps[0:1, :])
    nc.sync.dma_start(dscr[0:1, 0:128], warm_sb[0:1, :])

    # per-chunk tensors: q (reused as normalized o after p2), k, v-natural
    qT = [[sb.tile([128, 512], mdt, name=f"qT{c}_{h}") for h in range(HQ)] for c in range(NCHUNK)]
    kT = [sb.tile([128, 512], mdt, name=f"kT{c}") for c in range(NCHUNK)]
    vnat = [sb.tile([128, 512], mdt, name=f"vnat{c}") for c in range(NCHUNK)]

    ident = sb.tile([128, 128], mdt, name="ident")
    from concourse.masks import make_identity

    make_identity(nc, ident)


    # =================== phase 1: QKV projections + RoPE ===================
    def rope_inplace(dst, psrc, cct, sst):
        """dst[:, 0:512] = rope(psrc); partition rows 0:64 hold the even rope
        dims, 64:128 the odd ones (host permuted the weight rows)."""
        cpy = sbw.tile([128, 512], F32, tag="ropetmp", bufs=5, name="cpy")
        nc.scalar.copy(cpy[:], psrc[:])
        sw = sbw.tile([128, 512], F32, tag="ropetmp", bufs=5, name="sw")
        nc.scalar.copy(sw[0:64, :], cpy[64:128, :])
        nc.scalar.copy(sw[64:128, :], cpy[0:64, :])
        m1 = sbw.tile([128, 512], F32, tag="ropetmp", bufs=5, name="m1")
        m2 = sbw.tile([128, 512], F32, tag="ropetmp", bufs=5, name="m2")
        nc.gpsimd.tensor_mul(m1[:], cpy[:], cct[:])
        nc.gpsimd.tensor_mul(m2[:], sw[:], sst[:])
        nc.vector.tensor_sub(dst[0:64, :], m1[0:64, :], m2[0:64, :])
        nc.vector.tensor_add(dst[64:128, :], m1[64:128, :], m2[64:128, :])

    def p1_units(c, pair_order=((0, 1), (2, 3), (4, 5)), inject_weights=False):
        """Phase-1 filler closures for chunk c, as a list of per-pair unit
        lists. Outputs 0-3 are q heads, 4 is k, 5 is v. Each pair alternates
        two PSUM banks (same-bank back-to-back accumulation can't pipeline)."""
        s0 = c * 512
        # chunk-resident x: 8 tiles of 4 k-blocks [128, 4*512]
        xg = [None] * 8

        def load_xg(g):
            def run():
                xg[g] = sbw.tile([128, 4 * 512], mdt, tag="xt", bufs=12, name=f"xg{c}_{g}")
                nc.gpsimd.dma_start(
                    xg[g][:].rearrange("p (k s) -> p k s", k=4),
                    xT.rearrange("(k p) s -> p k s", p=128)[:, 4 * g : 4 * g + 4, s0 : s0 + 512],
                )
            return run

        cs = [None, None]

        def load_tbl():
            def run():
                cs[0] = sbw.tile([128, 512], F32, tag="tbl", bufs=4, name="cct")
                cs[1] = sbw.tile([128, 512], F32, tag="tbl", bufs=4, name="sst")
                nc.sync.dma_start(cs[0][:], cc_d[:, s0 : s0 + 512])
                nc.sync.dma_start(cs[1][:], ss_d[:, s0 : s0 + 512])
            return run

        state = {}

        def mk_mm(out_i, k):
            def run():
                if k == 0:
                    state[out_i] = ps.tile([128, 512], F32, tag="p1", bufs=2, name=f"p1acc{out_i}")
                if out_i < HQ:
                    wsl = wq_t[k // 4][:, (k % 4) * QC + out_i * 128 : (k % 4) * QC + (out_i + 1) * 128]
                elif out_i == HQ:
                    wsl = wk_t[k // 8][:, (k % 8) * HD : (k % 8 + 1) * HD]
                else:
                    wsl = wv_t[k // 8][:, (k % 8) * HD : (k % 8 + 1) * HD]
                xsl = xg[k // 4][:, (k % 4) * 512 : (k % 4 + 1) * 512]
                nc.tensor.matmul(state[out_i][:], wsl, xsl, start=(k == 0), stop=(k == NKBLK - 1))
            return run

        def mk_drain(out_i):
            def run():
                if out_i < HQ:
                    rope_inplace(qT[c][out_i], state[out_i], cs[0], cs[1])
                elif out_i == HQ:
                    rope_inplace(kT[c], state[out_i], cs[0], cs[1])
                else:
                    vtmp = sbw.tile([128, 512], mdt, tag="ropetmp", bufs=5, name="vtmp")
                    nc.scalar.copy(vtmp[:], state[out_i][:])
                    for i in range(4):
                        tp = ps.tile([128, 128], mdt, tag="p1", bufs=2, name="tp")
                        nc.tensor.transpose(tp[:], vtmp[:, i * 128 : (i + 1) * 128], ident)
                        nc.scalar.copy(vnat[c][:, i * 128 : (i + 1) * 128], tp[:])
            return run

        pair_lists = []
        loaded_xg = {0}
        for pi, (a, b) in enumerate(pair_order):
            units = []
            if pi == 0:
                units.append(load_xg(0))
                units.append(load_tbl())
            for k in range(NKBLK):
                if k % 4 == 0 and pi == 0:
                    for gg in (k // 4 + 1, k // 4 + 2):
                        if gg < 8 and gg not in loaded_xg:
                            loaded_xg.add(gg)
                            units.append(load_xg(gg))
                if inject_weights and pi == 0:
                    # chunk 0 streams weights just-in-time: wk/wv one 8-block
                    # tile ahead, wq tiles spread across the first pair.
                    if k % 8 == 0 and k // 8 + 1 < 4:
                        units.append(lambda g=k // 8 + 1: load_wk_wv(g))
                    if k % 4 == 2:
                        units.append(lambda g=k // 4: load_wq(g))
                units.append(mk_mm(a, k))
                units.append(mk_mm(b, k))
            units.append(mk_drain(a))
            units.append(mk_drain(b))
            pair_lists.append(units)
        return pair_lists

    # =================== phase 3: output projection ===================
    def p3_units(c, wide_banks=False):
        """Phase-3 filler closures for chunk c: 8 groups of 4 jb each. With
        wide_banks (p1 banks idle), rotate y_ps over 4 PSUM banks."""
        units = []

        def mk_group(jbg):
            def run():
                yst = sbw.tile([128, 4 * 512], mdt, tag="yst", bufs=2, name="yst")
                for jp in range(2):
                    tag = "p1" if (wide_banks and jp == 1) else "p3"
                    y_ps = {}
                    for jj in (2 * jp, 2 * jp + 1):
                        y_ps[jj] = ps.tile([128, 512], F32, tag=tag, bufs=2, name="y_ps")
                    for cb in range(4):
                        for jj in (2 * jp, 2 * jp + 1):
                            jb = jbg * 4 + jj
                            nc.tensor.matmul(
                                y_ps[jj][:],
                                wo_t[:, cb * HID + jb * 128 : cb * HID + (jb + 1) * 128],
                                qT[c][cb][:],
                                start=(cb == 0),
                                stop=(cb == 3),
                            )
                    for jj in (2 * jp, 2 * jp + 1):
                        if jj % 2 == 0:
                            nc.scalar.copy(yst[:, jj * 512 : (jj + 1) * 512], y_ps[jj][:])
                        else:
                            nc.vector.tensor_copy(yst[:, jj * 512 : (jj + 1) * 512], y_ps[jj][:])
                nc.sync.dma_start(
                    yT_d.rearrange("(jb p) s -> p jb s", p=128)[
                        :, jbg * 4 : jbg * 4 + 4, c * 512 : (c + 1) * 512
                    ],
                    yst[:].rearrange("p (jb s) -> p jb s", jb=4),
                )
            return run

        for jbg in range(8):
            units.append(mk_group(jbg))
        return units

    # =================== phase 2: attention ===================
    def p2_chunk(c, fillers):
        """Attention for chunk c, pulling filler units between j-iterations.
        Returns the last pair's drain units for the next chunk's stream."""
        carry = []
        jmax = 4 * c + 3
        n_slots = 2 * (jmax + 2)
        total = len(fillers)
        slot = [0, 0]  # slots done, units consumed

        def fill():
            slot[0] += 1
            target = (total * slot[0]) // n_slots if slot[0] < n_slots else total
            while slot[1] < target and fillers:
                fillers.pop(0)()
                slot[1] += 1

        for hp in range(HQ // 2):
            heads = (2 * hp, 2 * hp + 1)
            o_ps = {h: ps.tile([128, 512], F32, tag="o", bufs=2, name=f"o_ps{h}") for h in heads}
            den_acc = {}
            for h in heads:
                den_acc[h] = sbw.tile([128, 512], mdt, tag="den_acc", bufs=3, name=f"den_acc{h}")
            for j in range(jmax + 1):
                off = 128 * max(0, j - 4 * c)
                g = j - 4 * c
                st = j == 0
                sp = j == jmax
                jc, jb2 = j // 4, j % 4
                et = {}
                s_ps = {}
                for h in heads:
                    s_ps[h] = ps.tile([128, 512], F32, tag="s", bufs=2, name="s_ps")
                    nc.tensor.matmul(
                        s_ps[h][:, off:512],
                        kT[jc][:, jb2 * 128 : (jb2 + 1) * 128],
                        qT[c][h][:, off:512],
                        start=True,
                        stop=True,
                    )
                for h in heads:
                    et[h] = sbw.tile([128, 512], mdt, tag="et", bufs=10, name="et")
                    nc.scalar.activation(
                        et[h][:, off:512], s_ps[h][:, off:512],
                        mybir.ActivationFunctionType.Exp, scale=SCALE,
                    )
                    if g >= 0:  # diagonal block: keep keys kk <= s in block
                        nc.gpsimd.affine_select(
                            out=et[h][:, g * 128 : (g + 1) * 128],
                            in_=et[h][:, g * 128 : (g + 1) * 128],
                            compare_op=mybir.AluOpType.is_ge,
                            fill=0.0,
                            base=0,
                            pattern=[[1, 128]],
                            channel_multiplier=-1,
                        )
                for h in heads:
                    nc.tensor.matmul(
                        o_ps[h][:, off:512],
                        vnat[jc][:, jb2 * 128 : (jb2 + 1) * 128],
                        et[h][:, off:512],
                        start=st,
                        stop=sp,
                    )
                for h in heads:
                    # j == 0 always has off == 0, so the copy initializes all
                    # 512 columns of the accumulator.
                    if j == 0:
                        nc.vector.tensor_copy(den_acc[h][:], et[h][:])
                    else:
                        nc.vector.tensor_add(
                            den_acc[h][:, off:512],
                            den_acc[h][:, off:512],
                            et[h][:, off:512],
                        )
                fill()
            # drain: o_sb copies + den matmuls + reciprocal chain inline;
            # the broadcast matmul + normalize mul are deferred into the next
            # pair's / chunk's filler stream so the PE never waits on the
            # DVE reciprocal chain.
            o_sb = {}
            for i, h in enumerate(heads):
                o_sb[h] = sbw.tile([128, 512], F32, tag="osb", bufs=3, name="o_sb")
                if i % 2 == 0:
                    nc.vector.tensor_copy(o_sb[h][:], o_ps[h][:])
                else:
                    nc.scalar.copy(o_sb[h][:], o_ps[h][:])
            fill()
            rec = {}
            for h in heads:
                den_ps = ps.tile([128, 512], F32, tag="s", bufs=2, name="den_ps")
                nc.tensor.matmul(
                    den_ps[0:1, :], ones_b[:, 0:1], den_acc[h][:], start=True, stop=True
                )
                den_row = sbw.tile([1, 512], F32, tag="den", bufs=4, name="den_row")
                nc.vector.tensor_copy(den_row[0:1, :], den_ps[0:1, :])
                rec_f = sbw.tile([1, 512], F32, tag="den", bufs=4, name="rec_f")
                rec_scr = sbw.tile([1, 512], F32, tag="den", bufs=4, name="rec_scr")
                nc.vector.reciprocal_approx_accurate(
                    rec_f[0:1, :], den_row[0:1, :], rec_scr[0:1, :]
                )
                rec[h] = sbw.tile([1, 512], F32R, tag="den", bufs=4, name="rec_row")
                nc.vector.tensor_copy(rec[h][0:1, :], rec_f[0:1, :])

            def mk_u2(heads=heads, o_sb=o_sb, rec=rec):
                def run():
                    rec_ps = {}
                    for h in heads:
                        rec_ps[h] = ps.tile([128, 512], F32, tag="s", bufs=2, name="rec_ps")
                        nc.tensor.matmul(
                            rec_ps[h][:], ones[0:1, :], rec[h][0:1, :],
                            start=True, stop=True,
                        )
                    for h in heads:
                        # qT[c][h] becomes the normalized attention output
                        nc.vector.tensor_mul(qT[c][h][:], o_sb[h][:], rec_ps[h][:])
                return run

            if hp == 0:
                fillers.insert(5, mk_u2())
            else:
                carry.append(mk_u2())
        return carry

    # =================== pipeline driver ===================
    # p1(0): (k,v) and (q0,q1) run straight; (q2,q3) become leading fillers
    # for p2(0) so its first pair can start as soon as q0/q1 are roped.
    load_wk_wv(0)
    p10 = p1_units(0, pair_order=((4, 5), (0, 1), (2, 3)), inject_weights=True)
    for u in p10[0]:
        u()
    for u in p10[1]:
        u()
    carry = []
    for c in range(NCHUNK):
        fillers = []
        fillers += carry
        if c == 0:
            fillers += p10[2]
            fillers += wo_units()
        if c + 1 < NCHUNK:
            for pl in p1_units(c + 1):
                fillers += pl
        if c - 1 >= 0:
            fillers += p3_units(c - 1, wide_banks=(c - 1 >= 2))
        carry = p2_chunk(c, fillers)
        for u in fillers:  # anything the pacing didn't consume
            u()
    for u in carry:
        u()
    for u in p3_units(NCHUNK - 1, wide_banks=True):
        u()


_NC_CACHE = {}


def _get_nc():
    key = ("v2", MODE)
    if key not in _NC_CACHE:
        nc = bacc.Bacc("TRN2", target_bir_lowering=False, debug=False, num_devices=N_CORES)
        with TileContext(nc) as tc:
            with (
                tc.tile_pool(name="sb", bufs=1) as sb,
                tc.tile_pool(name="sbw", bufs=1) as sbw,
                tc.tile_pool(name="ps", bufs=1, space="PSUM") as ps,
            ):
                _build_body(tc, sb, sbw, ps)
        nc.compile()
        _NC_CACHE[key] = nc
    return _NC_CACHE[key]


_ROPE_PERM = np.concatenate([np.arange(0, 128, 2), np.arange(1, 128, 2)])


def _rope_tables(start_pos):
    freqs = 1.0 / (THETA ** (np.arange(0, HD, 2, dtype=np.float64) / HD))
    t = np.arange(start_pos, start_pos + SEQ, dtype=np.float64)
    ang = np.outer(t, freqs)  # [SEQ, 64]
    cosT = np.cos(ang).T.astype(np.float32)  # [64, SEQ]
    sinT = np.sin(ang).T.astype(np.float32)
    cc = np.ascontiguousarray(np.concatenate([cosT, cosT], axis=0))
    ss = np.ascontiguousarray(np.concatenate([sinT, sinT], axis=0))
    return cc, ss


def _to_kblock_layout(wT, cwidth):
    """[HID, cwidth] feature-major weight -> [128, NKBLK*cwidth] with k-block
    k at columns [k*cwidth, (k+1)*cwidth)."""
    return np.ascontiguousarray(
        wT.reshape(NKBLK, 128, cwidth).transpose(1, 0, 2).reshape(128, NKBLK * cwidth)
    )


def make_in_maps(x, wq, wk, wv, wo, start_pos):
    import ml_dtypes

    np_mdt = ml_dtypes.bfloat16
    x = np.asarray(x, dtype=np.float32)
    wq = np.asarray(wq, dtype=np.float32)
    wk = np.asarray(wk, dtype=np.float32)
    wv = np.asarray(wv, dtype=np.float32)
    wo = np.asarray(wo, dtype=np.float32)
    sp = int(start_pos)

    xT = np.ascontiguousarray(x.T).astype(np_mdt, copy=False)
    cc, ss = _rope_tables(sp)
    woT = np.ascontiguousarray(wo.T)  # [in=c, out=j]

    in_maps = []
    for c in range(N_CORES):
        wq_c = wq[c * QC : (c + 1) * QC, :]  # [512, HID]
        wq_c = wq_c.reshape(HQ, HD, HID)[:, _ROPE_PERM, :].reshape(QC, HID)
        wk_c = wk[c * HD : (c + 1) * HD, :][_ROPE_PERM, :]  # [128, HID]
        wv_c = wv[c * HD : (c + 1) * HD, :]  # [128, HID]
        wq_sbm = _to_kblock_layout(np.ascontiguousarray(wq_c.T), QC)
        wk_sbm = _to_kblock_layout(np.ascontiguousarray(wk_c.T), HD)
        wv_sbm = _to_kblock_layout(np.ascontiguousarray(wv_c.T), HD)
        woT_c = woT[c * QC : (c + 1) * QC, :]  # [512, HID]
        wo_sbm = np.ascontiguousarray(
            woT_c.reshape(4, 128, HID).transpose(1, 0, 2).reshape(128, 4 * HID)
        )
        in_maps.append(
            {
                "xT": xT,
                "wq_sb": wq_sbm.astype(np_mdt, copy=False),
                "wk_sb": wk_sbm.astype(np_mdt, copy=False),
                "wv_sb": wv_sbm.astype(np_mdt, copy=False),
                "wo_sb": wo_sbm.astype(np_mdt, copy=False),
                "cc": cc,
                "ss": ss,
            }
        )
    return in_maps


def _assemble(results):
    acc = results[0]["yT"].astype(np.float32)
    for r in results[1:]:
        acc = acc + r["yT"].astype(np.float32)
    return np.ascontiguousarray(acc.T)


def _row0_expected(x, wv, wo):
    """Exact y[0]: query 0 attends only key 0, so o[0] is v[0] broadcast over
    the 4 q heads of each kv head; cheap host-side corruption check."""
    v0 = np.asarray(x[0], np.float64) @ np.asarray(wv, np.float64).T  # [1024]
    o0 = np.concatenate([v0[(h // HQ) * HD : (h // HQ + 1) * HD] for h in range(NQ)])
    return o0 @ np.asarray(wo, np.float64).T  # [4096]


def kernel(x, wq, wk, wv, wo, start_pos):
    nc = _get_nc()
    in_maps = make_in_maps(x, wq, wk, wv, wo, start_pos)
    y0 = _row0_expected(x, wv, wo)
    out = None
    for attempt in range(2):
        res = bass_utils.run_bass_kernel_spmd(nc, in_maps, core_ids=list(range(N_CORES)))
        out = _assemble(res.results)
        err0 = float(np.linalg.norm(out[0] - y0) / (np.linalg.norm(y0) + 1e-30))
        if np.isfinite(out).all() and err0 < 2e-2:
            break
        # a wedged device can corrupt a run silently; one retry clears it
    return out
